# revision 47
# baseline (speedup 1.0000x reference)
"""Trainium2 Bass kernel for a dense transformer block (nn_Block_88338887344891).

Distribution over 8 NeuronCores (single SPMD NEFF, 2 AllToAll collectives):
  - LN1 stats computed fully locally per core from its fp8 copy of x (no
    collective): x^2 on ACT+Pool, sums as fp8-DoubleRow matmuls into one
    psum bank (rows 0/32), stats chain runs two chunks ahead of use.
  - LayerNorm folded into the QKV matmuls: QKV = inv * (W'^T x + s (-mu)) + b'
    where W' = diag(ln_w) W (host-folded), s = colsum(W'); raw DoubleRow
    matmuls run one chunk ahead of the rank-1 corrections.
  - QKV + causal attention head-sharded (2 heads/core over all 4096 tokens);
    scores/exp/AV trimmed to the causal region; V produced directly in
    keys-major fp8; softmax probs exp'd to fp8 (bias -2 keeps e^x in range,
    cancels in the ratio); AV matmuls fp8-DoubleRow over key-tile pairs.
  - attention inner loop software-pipelined (scores k+1 before AV k) so the
    exp (ACT) latency never blocks the PE; ACT runs at ~96% in attention.
  - attention output AllToAll per head in fp8 (21.5us each; #0 hidden under
    h1 compute, #1 exposed); readbacks on the Pool queue.
  - output projection + residual + LN2 (sums accumulated inside the Wo loop,
    rsqrt via DVE ALU pow, no ACT table swap) + full MLP token-sharded.
  - gelu(tanh approx) via t*sigmoid(1.702 t) (1 ACT + 1 fused DVE op).
PSUM accumulates f32; 16x weight scaling with fp8 hi+lo splitting."""
import numpy as np
from contextlib import ExitStack

try:  # persistent XLA cache so repeat runs skip the NEFF compile
    import jax
    jax.config.update("jax_compilation_cache_dir", "/tmp/jax_neff_cache")
    jax.config.update("jax_persistent_cache_min_compile_time_secs", 1.0)
except Exception:
    pass

import ml_dtypes
import concourse.bass as bass
import concourse.bacc as bacc
import concourse.tile as tile
import concourse.mybir as mybir
from concourse import bass_utils

AF = mybir.ActivationFunctionType
ALU = mybir.AluOpType
F32 = mybir.dt.float32
F32R = mybir.dt.float32r
BF16 = mybir.dt.bfloat16
FP8 = mybir.dt.float8e4
NPBF16 = ml_dtypes.bfloat16
NPFP8 = mybir.dt.np(mybir.dt.float8e4)
DR = mybir.MatmulPerfMode.DoubleRow

NC_N = 8          # cores
B, T, D, H = 2, 2048, 1024, 16
HD = D // H       # 64
DFF = 4 * D       # 4096
EPS = 1e-5
BT = B * T               # 4096 tokens
TPC = BT // NC_N         # 512 tokens per core
HPC = H // NC_N          # 2 heads per core
PO = D // 128            # 8 D-tiles
M1 = DFF // 128          # 32 ff1 out tiles
NKT = BT // 128          # 32 key tiles globally (16 per batch)
RG = [list(range(NC_N))]
SIGC = 1.702             # gelu sigmoid-form constant

GELU_NATIVE = False   # kept for test.py compat; kernel is sim/hw identical

# Wo row order after the per-head AllToAll halves: feature index
# n = half*512 + po*128 + s2*64 + d maps to old row 64*(2*(2*po+s2)+half)+d
WO_PERM = np.array([64 * (2 * (2 * po + s2) + half) + d
                    for half in range(2) for po in range(4)
                    for s2 in range(2) for d in range(64)])

_CACHE = {}


def _build():
    nc = bacc.Bacc("TRN2", target_bir_lowering=False, debug=False,
                   num_devices=NC_N)

    # ---- per-core external inputs ----
    xb_in = nc.dram_tensor("xb", [D, BT], FP8, kind="ExternalInput")
    xf_in = nc.dram_tensor("xf", [D, TPC], F32, kind="ExternalInput")
    wqkv_in = nc.dram_tensor("wqkv", [D, 768], FP8, kind="ExternalInput")
    # packed constants: cb (bf16 row), cf (f32r row), cp (per-partition f32)
    cb_in = nc.dram_tensor("cb", [1, 2304], BF16, kind="ExternalInput")
    cf_in = nc.dram_tensor("cf", [1, 512], F32R, kind="ExternalInput")
    cp_in = nc.dram_tensor("cp", [128, 2 * M1], F32, kind="ExternalInput")
    wo_in = nc.dram_tensor("wo", [D, 2 * D], FP8, kind="ExternalInput")
    wf1_in = nc.dram_tensor("wf1", [D, 2 * DFF], FP8, kind="ExternalInput")
    wf2_in = nc.dram_tensor("wf2", [DFF, 2 * D], FP8, kind="ExternalInput")
    out_t = nc.dram_tensor("outt", [D, TPC], F32, kind="ExternalOutput")

    with tile.TileContext(nc, pool_alloc_mode="queue") as tc, \
            ExitStack() as ctx:
        perm = ctx.enter_context(tc.tile_pool(name="perm", bufs=1))
        big = ctx.enter_context(tc.tile_pool(name="big", bufs=1))
        rows = ctx.enter_context(tc.tile_pool(name="rows", bufs=1))
        dram = ctx.enter_context(tc.tile_pool(name="dram", bufs=1, space="DRAM"))

        # ---- constants ----
        ones2 = perm.tile([128, 2, 1], FP8)   # DR stats-sum stationary
        nc.vector.memset(ones2[:], 1.0)
        ones_col_f = perm.tile([128, 1], F32)
        nc.vector.memset(ones_col_f[:], 1.0)
        ones_col_r = perm.tile([128, 1], F32R)
        nc.vector.tensor_copy(ones_col_r[:], ones_col_f[:])
        ones_row_f = perm.tile([1, 128], F32)
        nc.vector.memset(ones_row_f[:], 1.0)
        ones_row_r = perm.tile([1, 128], F32R)
        nc.vector.tensor_copy(ones_row_r[:], ones_row_f[:])
        ones_tok = perm.tile([1, TPC], BF16)
        nc.vector.memset(ones_tok[:], 1.0)
        ebias = perm.tile([128, 1], F32)   # exp range bias for fp8 probs
        nc.vector.memset(ebias[:], -2.0)

        # packed constants (single DMA each; issued after the big x/w loads
        # below to keep the HWDGE path clear at startup)
        cb = perm.tile([1, 2304], BF16, tag="c_cb")
        cf = perm.tile([1, 512], F32R, tag="c_cf")
        cp = perm.tile([128, 2 * M1], F32, tag="c_cp")
        sv_row = cb[:, 0:128]
        bv_row = cb[:, 128:256]
        bo_r = cb[:, 256:256 + D]        # 16*b_o
        bf2_r = cb[:, 256 + D:256 + 2 * D]   # 256*b_ff2
        sqkv = cf[:, 0:256]
        bqkr = cf[:, 256:512]
        bf1 = cp[:, 0:M1]                # 16*b_ff1
        bf1s = cp[:, M1:2 * M1]          # SIGC/16 bias

        # ---- persistent SBUF ----
        invc = big.tile([128, NKT], F32R, tag="invc")  # inv keys-major
        X2 = big.tile([128, PO, TPC], F32R, tag="x2")
        xh2 = big.tile([128, PO, TPC], FP8, tag="xh2")
        Amat = big.tile([128, M1, TPC], FP8, tag="amat")

        # dram scratch
        invrt = dram.tile([NC_N, TPC], F32R)   # inv row -> keys-major hop
        a2ai0 = dram.tile([NC_N, 64, TPC], FP8)
        a2ao0 = dram.tile([NC_N, 64, TPC], FP8)
        a2ai1 = dram.tile([NC_N, 64, TPC], FP8)
        a2ao1 = dram.tile([NC_N, 64, TPC], FP8)

        xb_view = xb_in.ap().rearrange("(po p) (k t) -> k p po t",
                                       p=128, t=TPC)

        xlp_cm = tc.tile_pool(name="xlp", bufs=1)
        xlp = xlp_cm.__enter__()
        xl = xlp.tile([128, PO, TPC], F32, tag="xl")   # my x (residual)
        wo_sb = xlp.tile([128, PO, 2 * D], FP8, tag="wo")
        qkvp_cm = tc.tile_pool(name="qkvp", bufs=1)
        qkvp = qkvp_cm.__enter__()
        Qh = qkvp.tile([128, BT], BF16, tag="qh")      # 2 heads stacked
        Kh = qkvp.tile([128, BT], BF16, tag="kh")
        Vt = qkvp.tile([128, HPC, NKT, 65], FP8, tag="vt")  # keys-major V
        nc.gpsimd.memset(Vt[:, :, :, 64:65], 1.0)   # softmax denominator row
        tri = perm.tile([128, 128], FP8, tag="tri")  # tri[p,q]=1 iff q>=p
        nc.gpsimd.memset(tri[:], 1.0)
        nc.gpsimd.affine_select(
            out=tri[:], in_=tri[:], compare_op=ALU.is_ge, fill=0.0,
            base=0, pattern=[[1, 128]], channel_multiplier=-1)

        # ===== Phases 1+2: fully local LN1 stats + QKV, one fused pass ====
        # Every core computes all 8 chunks' stats from its own fp8 copy of
        # x (no collective): squares on ACT+Pool, sums as fp8-DR matmuls.
        # Raw-G DoubleRow matmuls run one chunk ahead of the rank-1 LN
        # corrections (s (x) -mu, b (x) std).
        invrt_v = invrt[:].rearrange("c (kt p) -> c p kt", p=128)
        with tc.tile_pool(name="wq", bufs=1) as wq, \
             tc.tile_pool(name="xcp", bufs=2) as xcp, \
             tc.tile_pool(name="sqp", bufs=3) as sqp, \
             tc.tile_pool(name="strp", bufs=2) as strp, \
             tc.tile_pool(name="psA", bufs=4, space="PSUM") as psA, \
             tc.tile_pool(name="psBb", bufs=1, space="PSUM") as psbp, \
             tc.tile_pool(name="psSt", bufs=1, space="PSUM") as pstp, \
             tc.tile_pool(name="psV", bufs=2, space="PSUM") as psVt:
            # one psum bank holds all per-chunk stat rows: chunk parity
            # selects partition pair (0,32) or (64,96), so two chunks'
            # stats can be in flight in a single bank
            pstat2 = pstp.tile([128, TPC], F32)
            wqkv_sb = wq.tile([128, PO, 768], FP8)
            wqkv_v = wqkv_in.ap().rearrange("(po p) m -> p po m", p=128)
            nc.sync.dma_start(wqkv_sb[:, 0:PO // 2, :], wqkv_v[:, 0:PO // 2, :])
            nc.sync.dma_start(wqkv_sb[:, PO // 2:PO, :],
                              wqkv_v[:, PO // 2:PO, :])
            xc0 = xcp.tile([128, PO, TPC], FP8, tag="xc")
            nc.sync.dma_start(xc0[:], xb_view[0])

            raw_state = {}
            stat_state = {}

            def emit_raw(c):
                if c == 0:
                    xc = xc0
                else:
                    xc = xcp.tile([128, PO, TPC], FP8, tag="xc",
                                  name=f"xc_{c}")
                    nc.sync.dma_start(xc[:], xb_view[c])
                sqs = []
                for j in range(4):   # x^2 pair-tiles: 2 on ACT, 2 on Pool
                    sq = sqp.tile([128, 2, TPC], FP8, tag="sq",
                                  name=f"sq_{c}_{j}")
                    pj = slice(2 * j, 2 * j + 2)
                    # chunk 0: all squares on Pool so the ACT table load +
                    # square latency doesn't delay the first stat chain
                    if j < 2 and c > 0:
                        nc.scalar.activation(sq[:], xc[:, pj, :], AF.Square)
                    else:
                        nc.gpsimd.tensor_mul(sq[:], xc[:, pj, :],
                                             xc[:, pj, :])
                    sqs.append(sq)
                if c == 0:   # consts go behind chunk 0's Pool squares
                    nc.gpsimd.dma_start(cb[:], cb_in.ap())
                    nc.gpsimd.dma_start(cf[:], cf_in.ap())
                    nc.gpsimd.dma_start(cp[:], cp_in.ap())
                qk_ps = []
                for m in range(2):
                    ps = psA.tile([128, TPC], F32, tag="ps",
                                  name=f"psqk_{c}_{m}")
                    chi = slice(128 * m, 128 * m + 128)
                    clo = slice(384 + 128 * m, 384 + 128 * m + 128)
                    for k in range(PO // 2):
                        ksl = slice(2 * k, 2 * k + 2)
                        nc.tensor.matmul(ps[:], wqkv_sb[:, ksl, chi],
                                         xc[:, ksl, :], start=(k == 0),
                                         stop=False, perf_mode=DR)
                        nc.tensor.matmul(ps[:], wqkv_sb[:, ksl, clo],
                                         xc[:, ksl, :], start=False,
                                         stop=False, perf_mode=DR)
                    qk_ps.append(ps)
                psv = psVt.tile([128, TPC], F32, tag="psv",
                                name=f"psv_{c}")
                # V in keys-major layout: 4 column pieces share one psum
                # bank = one accumulation group (start zeroes the full 2KB
                # zero region; the last V correction in emit_corr stops it)
                for kt in range(4):
                    vsl = slice(128 * kt, 128 * kt + 128)
                    for k in range(PO // 2):
                        ksl = slice(2 * k, 2 * k + 2)
                        nc.tensor.matmul(
                            psv[:, vsl], xc[:, ksl, vsl],
                            wqkv_sb[:, ksl, 256:384],
                            start=(kt == 0 and k == 0),
                            stop=False, perf_mode=DR)
                        nc.tensor.matmul(
                            psv[:, vsl], xc[:, ksl, vsl],
                            wqkv_sb[:, ksl, 640:768], start=False,
                            stop=False, perf_mode=DR)
                raw_state[c] = (qk_ps, psv, xc, sqs)

            def emit_stats(c):
                _, _, xc, sqs = raw_state[c]
                po_s = 0                     # sum row partition
                po_q = 32                    # sumsq row partition
                for k in range(4):
                    nc.tensor.matmul(pstat2[po_s:po_s + 1, :], ones2[:],
                                     xc[:, 2 * k:2 * k + 2, :],
                                     start=(k == 0), stop=(k == 3),
                                     perf_mode=DR)
                for j in range(4):
                    nc.tensor.matmul(pstat2[po_q:po_q + 1, :], ones2[:],
                                     sqs[j][:],
                                     start=(j == 0), stop=(j == 3),
                                     perf_mode=DR)
                murow = strp.tile([1, TPC], F32R, tag="mur",
                                  name=f"mur_{c}")
                nc.scalar.activation(murow[:].bitcast(F32),
                                     pstat2[po_s:po_s + 1, :],
                                     AF.Copy, scale=-1.0 / D)     # -mu
                ex2 = strp.tile([1, TPC], F32, tag="ex2", name=f"ex2_{c}")
                nc.scalar.activation(ex2[:], pstat2[po_q:po_q + 1, :],
                                     AF.Copy, scale=1.0 / D)
                mu2 = strp.tile([1, TPC], F32, tag="mu2", name=f"mu2_{c}")
                nc.scalar.activation(mu2[:], murow[:].bitcast(F32),
                                     AF.Square)
                var = strp.tile([1, TPC], F32, tag="var", name=f"var_{c}")
                nc.vector.scalar_tensor_tensor(
                    out=var[:], in0=ex2[:], scalar=EPS, in1=mu2[:],
                    op0=ALU.add, op1=ALU.subtract)
                stdrow = strp.tile([1, TPC], F32R, tag="std",
                                   name=f"std_{c}")
                nc.vector.tensor_scalar(out=stdrow[:].bitcast(F32),
                                        in0=var[:], scalar1=0.5,
                                        scalar2=None, op0=ALU.pow)
                invrow = strp.tile([1, TPC], F32R, tag="ivr",
                                   name=f"ivr_{c}")
                nc.vector.tensor_scalar(out=invrow[:].bitcast(F32),
                                        in0=var[:], scalar1=-0.5,
                                        scalar2=None, op0=ALU.pow)
                mub = strp.tile([1, TPC], BF16, tag="mub", name=f"mub_{c}")
                nc.vector.tensor_copy(mub[:], murow[:])
                stdb = strp.tile([1, TPC], BF16, tag="stdb",
                                 name=f"stdb_{c}")
                nc.vector.tensor_copy(stdb[:], stdrow[:])
                # inv -> keys-major invc via a DRAM hop
                nc.sync.dma_start(invrt[c:c + 1, :], invrow[:])
                nc.sync.dma_start(invc[:, 4 * c:4 * c + 4], invrt_v[c])
                stat_state[c] = (murow, invrow, stdrow, mub, stdb)

            def emit_corr(c):
                qk_ps, psv, xc, sqs = raw_state.pop(c)
                murow, invrow, stdrow, mub, stdb = stat_state.pop(c)
                tok = slice(TPC * c, TPC * (c + 1))
                # inv broadcast; read straight out of PSUM by the evacs
                psb = psbp.tile([128, TPC], F32, tag="psb", name=f"psb_{c}")
                nc.tensor.matmul(psb[:], ones_row_r[:], invrow[:],
                                 start=True, stop=True)
                for m in range(2):
                    ps = qk_ps[m]
                    # ps holds 16*G: corrections are scaled by 16 to match
                    nc.tensor.matmul(ps[:], sqkv[:, 128 * m:128 * m + 128],
                                     murow[:], start=False, stop=False)
                    nc.tensor.matmul(ps[:], bqkr[:, 128 * m:128 * m + 128],
                                     stdrow[:], start=False, stop=True)
                nc.vector.scalar_tensor_tensor(
                    out=Qh[:, tok], in0=qk_ps[0][:], scalar=1.0 / 16.0,
                    in1=psb[:], op0=ALU.mult, op1=ALU.mult)
                nc.vector.scalar_tensor_tensor(
                    out=Kh[:, tok], in0=qk_ps[1][:], scalar=1.0 / 16.0,
                    in1=psb[:], op0=ALU.mult, op1=ALU.mult)
                for kt in range(4):
                    ksl = slice(128 * kt, 128 * kt + 128)
                    nc.tensor.matmul(psv[:, ksl], mub[:, ksl], sv_row[:],
                                     start=False, stop=False)
                    nc.tensor.matmul(psv[:, ksl], stdb[:, ksl], bv_row[:],
                                     start=False, stop=(kt == 3))
                for kt in range(4):
                    g = 4 * c + kt
                    nc.vector.tensor_scalar(
                        out=Vt[:, 0:HPC, g, 0:64],
                        in0=psv[:, 128 * kt:128 * kt + 128],
                        scalar1=1.0 / 16.0,
                        scalar2=invc[:, g:g + 1].bitcast(F32),
                        op0=ALU.mult, op1=ALU.mult)

            # stats run two chunks ahead of corrections so the per-chunk
            # ACT/DVE stat chain latency never blocks the PE
            emit_raw(0)
            emit_stats(0)
            emit_raw(1)
            emit_stats(1)
            for c in range(NC_N):
                emit_corr(c)
                if c + 2 < NC_N:
                    emit_raw(c + 2)
                    emit_stats(c + 2)

            # residual x + Wo weights: needed from phase 4 on
            nc.sync.dma_start(xl[:],
                              xf_in.ap().rearrange("(po p) t -> p po t", p=128))
            nc.sync.dma_start(
                wo_sb[:], wo_in.ap().rearrange("(po p) n -> p po n", p=128))

        # ============ Phase 3: causal attention (sw-pipelined) ============
        # scores for step k+1 are emitted before the AV matmul of step k so
        # the PE never sits behind the exp (ACT) of the current step; the
        # denominator broadcast reuses the group's own ps_av tile in place.
        with tc.tile_pool(name="ptp", bufs=4) as ptp, \
             tc.tile_pool(name="rcd", bufs=2) as rcdp, \
             tc.tile_pool(name="avp", bufs=2) as avp, \
             tc.tile_pool(name="psS", bufs=2, space="PSUM") as psS, \
             tc.tile_pool(name="psV2", bufs=2, space="PSUM") as psV2:
            a2a_ins = [a2ai0, a2ai1]
            for h in range(HPC):
                hsl = slice(64 * h, 64 * h + 64)
                groups = [(b, qh, 8 if qh == 0 else 16)
                          for b in range(B) for qh in range(2)]
                steps = [(gi, i) for gi, (b, qh, n_i) in enumerate(groups)
                         for i in range(n_i)]
                ps_avs = {}
                pts = {}

                def av_ops(qh, n_i):
                    # (lo, hi, dr, pair) AV-matmul regions in emit order;
                    # plane-0 solo over the diagonal gap, fp8-DR elsewhere
                    ops = []
                    for g in range(n_i // 2):
                        off0 = max(0, 128 * 2 * g - 1024 * qh)
                        off1 = max(0, 128 * (2 * g + 1) - 1024 * qh)
                        if off1 > off0:
                            ops.append((off0, off1, False, g))
                        lo = off1
                        while lo < 1024:
                            hi = min(1024, (lo // 512 + 1) * 512)
                            ops.append((lo, hi, True, g))
                            lo = hi
                    first = {}
                    last = {}
                    for idx, (lo, hi, dr, g) in enumerate(ops):
                        hb = lo // 512
                        first.setdefault(hb, idx)
                        last[hb] = idx
                    return ops, first, last

                def emit_scores(gi, i, h=h, hsl=hsl, groups=groups,
                                ps_avs=ps_avs, pts=pts):
                    b, qh, n_i = groups[gi]
                    if i == 0:
                        ps_avs[gi] = psV2.tile([65, 1024], F32, tag="psav",
                                               name=f"psav_{h}_{gi}")
                    koff = 2048 * b + 128 * i
                    qlo = max(1024 * qh, 128 * i)
                    free = 1024 * (qh + 1) - qlo
                    off = qlo - 1024 * qh
                    qabs = 2048 * b + qlo
                    spieces = []  # pss-aligned; stay within one psum bank
                    lo = 0
                    while lo < free:
                        hi = min(free, (lo // 512 + 1) * 512)
                        spieces.append((lo, hi - lo))
                        lo = hi
                    pss = psS.tile([128, 1024], F32, tag="pss")
                    for (plo, pfree) in spieces:
                        qa = qabs + plo
                        nc.tensor.matmul(
                            pss[:, plo:plo + pfree],
                            Kh[hsl, koff:koff + 128],
                            Qh[hsl, qa:qa + pfree],
                            start=True, stop=True)
                    if i % 2 == 0:   # fp8 pair tile, planes at fixed q-base
                        pt = ptp.tile([128, 2, 1024], FP8, tag="pt",
                                      name=f"pt_{h}_{gi}_{i}")
                        pts[(gi, i // 2)] = pt
                    else:
                        pt = pts[(gi, i // 2)]
                    pl = i % 2
                    nc.scalar.activation(pt[:, pl, off:off + free],
                                         pss[:, 0:free],
                                         AF.Exp, scale=0.125, bias=ebias[:])
                    if 128 * i >= 1024 * qh:   # diagonal tile
                        if h == 0:
                            nc.gpsimd.affine_select(
                                out=pt[:, pl, off:off + 128],
                                in_=pt[:, pl, off:off + 128],
                                compare_op=ALU.is_ge, fill=0.0,
                                base=0, pattern=[[1, 128]],
                                channel_multiplier=-1)
                        else:
                            nc.vector.tensor_mul(pt[:, pl, off:off + 128],
                                                 pt[:, pl, off:off + 128],
                                                 tri[:])

                def emit_av(gi, g, h=h, groups=groups,
                            ps_avs=ps_avs, pts=pts):
                    b, qh, n_i = groups[gi]
                    ops, first, last = av_ops(qh, n_i)
                    pt = pts.pop((gi, g))
                    kt0 = 16 * b + 2 * g
                    for idx, (lo, hi, dr, g2) in enumerate(ops):
                        if g2 != g:
                            continue
                        hb = lo // 512
                        if dr:
                            nc.tensor.matmul(
                                ps_avs[gi][:, lo:hi],
                                Vt[:, h, kt0:kt0 + 2, :],
                                pt[:, :, lo:hi],
                                start=(idx == first[hb]),
                                stop=(idx == last[hb]), perf_mode=DR)
                        else:
                            nc.tensor.matmul(
                                ps_avs[gi][:, lo:hi],
                                Vt[:, h, kt0, :],
                                pt[:, 0, lo:hi],
                                start=(idx == first[hb]),
                                stop=(idx == last[hb]))

                def emit_epilogue(gi, h=h, groups=groups, ps_avs=ps_avs):
                    b, qh, n_i = groups[gi]
                    ps_av = ps_avs.pop(gi)
                    recd = rcdp.tile([1, 1024], F32R, tag="recd")
                    nc.vector.reciprocal(recd[:].bitcast(F32),
                                         ps_av[64:65, :])
                    avs = avp.tile([65, 1024], F32, tag="avs")
                    nc.vector.tensor_copy(avs[0:64, :], ps_av[0:64, :])
                    for half in range(2):   # in-place denom broadcast
                        nc.tensor.matmul(
                            ps_av[0:64, 512 * half:512 * half + 512],
                            ones_row_r[:, 0:64],
                            recd[:, 512 * half:512 * half + 512],
                            start=True, stop=True)
                    avn = avp.tile([64, 1024], FP8, tag="avn")
                    nc.vector.tensor_mul(avn[:], avs[0:64, :],
                                         ps_av[0:64, :])
                    g0 = 4 * b + 2 * qh
                    nc.sync.dma_start(a2a_ins[h][g0, :, :], avn[:, 0:TPC])
                    nc.sync.dma_start(a2a_ins[h][g0 + 1, :, :],
                                      avn[:, TPC:2 * TPC])

                emit_scores(*steps[0])
                for k, (gi, i) in enumerate(steps):
                    if k + 1 < len(steps):
                        emit_scores(*steps[k + 1])
                    if i % 2 == 1:
                        emit_av(gi, i // 2)
                    if i == groups[gi][2] - 1:
                        emit_epilogue(gi)
                if h == 0:      # launch early; h=1's goes after the
                    nc.gpsimd.collective_compute(   # pool scope closes
                        "AllToAll", ALU.bypass, replica_groups=RG,
                        ins=[a2ai0[:].opt()], outs=[a2ao0[:].opt()])

        qkvp_cm.__exit__(None, None, None)   # free Qh/Kh/Vt

        # ==== Phases 4-6 in one pool scope: everything here is emitted
        # before the second AllToAll so pool-open barriers don't serialize
        # on it; Wo's first half + weight prefetches overlap the collective.
        with tc.tile_pool(name="avtp", bufs=1) as avtp, \
             tc.tile_pool(name="w1p", bufs=2) as w1p, \
             tc.tile_pool(name="w2p", bufs=2) as w2p, \
             tc.tile_pool(name="tmp2", bufs=2) as tmp2p, \
             tc.tile_pool(name="sgp", bufs=2) as sgp, \
             tc.tile_pool(name="outp", bufs=2) as outp, \
             tc.tile_pool(name="psA2", bufs=8, space="PSUM") as psA2:
            psB = psA2
            w1_sb = []
            w2_sb = []
            for g in range(4):   # prefetch during the A2A / Wo phase
                w1t = w1p.tile([128, PO, 2048], FP8, tag="w1")
                nc.sync.dma_start(
                    w1t[:, :, 0:1024], wf1_in.ap()[:, 1024 * g:1024 * (g + 1)]
                    .rearrange("(po p) n -> p po n", p=128))
                nc.sync.dma_start(
                    w1t[:, :, 1024:2048],
                    wf1_in.ap()[:, DFF + 1024 * g:DFF + 1024 * (g + 1)]
                    .rearrange("(po p) n -> p po n", p=128))
                w1_sb.append(w1t)
            for g in range(4):
                w2t = w2p.tile([128, M1, 512], FP8, tag="w2")
                nc.sync.dma_start(
                    w2t[:], wf2_in.ap()[:, 512 * g:512 * (g + 1)]
                    .rearrange("(ko p) n -> p ko n", p=128))
                w2_sb.append(w2t)

            AVt0 = avtp.tile([128, 4, TPC], FP8, tag="avt0")
            nc.gpsimd.dma_start(
                AVt0[:],
                a2ao0[:].rearrange("(po s2) p t -> (s2 p) po t", s2=2))
            wo_ps = []
            for m in range(PO):   # h0 half: overlaps the second AllToAll
                ps = psA2.tile([128, TPC], F32, tag="ps")
                chi = slice(128 * m, 128 * m + 128)
                clo = slice(D + 128 * m, D + 128 * m + 128)
                for k in range(2):
                    ksl = slice(2 * k, 2 * k + 2)
                    nc.tensor.matmul(ps[:], wo_sb[:, ksl, chi],
                                     AVt0[:, ksl, :], start=(k == 0),
                                     stop=False, perf_mode=DR)
                    nc.tensor.matmul(ps[:], wo_sb[:, ksl, clo],
                                     AVt0[:, ksl, :], start=False,
                                     stop=False, perf_mode=DR)
                wo_ps.append(ps)

            nc.gpsimd.collective_compute(
                "AllToAll", ALU.bypass, replica_groups=RG,
                ins=[a2ai1[:].opt()], outs=[a2ao1[:].opt()])
            AVt1 = avtp.tile([128, 4, TPC], FP8, tag="avt1")
            nc.gpsimd.dma_start(
                AVt1[:],
                a2ao1[:].rearrange("(po s2) p t -> (s2 p) po t", s2=2))

            ln2ps = None
            sq2s = []
            for m in range(PO):   # h1 half + epilogue + inline LN2 sums
                ps = wo_ps[m]
                chi = slice(128 * m, 128 * m + 128)
                clo = slice(D + 128 * m, D + 128 * m + 128)
                for k in range(2):
                    ksl = slice(2 * k, 2 * k + 2)
                    nc.tensor.matmul(ps[:], wo_sb[:, 4 + 2 * k:6 + 2 * k, chi],
                                     AVt1[:, ksl, :], start=False,
                                     stop=False, perf_mode=DR)
                    nc.tensor.matmul(ps[:], wo_sb[:, 4 + 2 * k:6 + 2 * k, clo],
                                     AVt1[:, ksl, :], start=False,
                                     stop=False, perf_mode=DR)
                nc.tensor.matmul(ps[:], bo_r[:, 128 * m:128 * m + 128],
                                 ones_tok[:], start=False, stop=True)
                nc.vector.scalar_tensor_tensor(
                    out=X2[:, m, :].bitcast(F32), in0=ps[:],
                    scalar=1.0 / 16.0, in1=xl[:, m, :],
                    op0=ALU.mult, op1=ALU.add)
                sq = tmp2p.tile([128, TPC], F32R, tag="sq2",
                                name=f"sq2_{m}")
                nc.scalar.activation(sq[:].bitcast(F32),
                                     X2[:, m, :].bitcast(F32), AF.Square)
                sq2s.append(sq)
                if m == 0:   # takes the bank wo_ps[0] just released
                    ln2ps = psB.tile([128, TPC], F32, tag="ps")
                else:   # stats lag one tile so DVE/ACT latency stays hidden
                    nc.tensor.matmul(ln2ps[0:1, :], ones_col_r[:],
                                     X2[:, m - 1, :],
                                     start=(m == 1), stop=False)
                    nc.tensor.matmul(ln2ps[32:33, :], ones_col_r[:],
                                     sq2s[m - 1][:],
                                     start=(m == 1), stop=False)
            nc.tensor.matmul(ln2ps[0:1, :], ones_col_r[:],
                             X2[:, PO - 1, :], start=False, stop=True)
            nc.tensor.matmul(ln2ps[32:33, :], ones_col_r[:],
                             sq2s[PO - 1][:], start=False, stop=True)
            nmu2 = rows.tile([1, TPC], F32R, tag="nmu2")
            nc.vector.tensor_scalar_mul(nmu2[:].bitcast(F32),
                                        ln2ps[0:1, :], -1.0 / D)
            ex2b = rows.tile([1, TPC], F32, tag="ex2b")
            nc.vector.tensor_scalar_mul(ex2b[:], ln2ps[32:33, :], 1.0 / D)
            mu2b = rows.tile([1, TPC], F32, tag="mu2b")
            nc.scalar.activation(mu2b[:], nmu2[:].bitcast(F32), AF.Square)
            varb = rows.tile([1, TPC], F32, tag="varb")
            nc.vector.scalar_tensor_tensor(
                out=varb[:], in0=ex2b[:], scalar=EPS, in1=mu2b[:],
                op0=ALU.add, op1=ALU.subtract)
            inv2 = rows.tile([1, TPC], F32R, tag="inv2")
            nc.vector.tensor_scalar(out=inv2[:].bitcast(F32), in0=varb[:],
                                    scalar1=-0.5, scalar2=None, op0=ALU.pow)
            ps_mu = psB.tile([128, TPC], F32, tag="ps")
            nc.tensor.matmul(ps_mu[:], ones_row_r[:], nmu2[:],
                             start=True, stop=True)
            ps_iv = psB.tile([128, TPC], F32, tag="ps")
            nc.tensor.matmul(ps_iv[:], ones_row_r[:], inv2[:],
                             start=True, stop=True)
            for po in range(PO):
                t0 = tmp2p.tile([128, TPC], F32, tag="t0")
                nc.vector.tensor_add(t0[:], X2[:, po, :].bitcast(F32),
                                     ps_mu[:])
                nc.vector.tensor_mul(xh2[:, po, :], t0[:], ps_iv[:])

            for m in range(M1):
                ps = psB.tile([128, TPC], F32, tag="ps")
                w1t = w1_sb[m // 8]
                chi = slice(128 * (m % 8), 128 * (m % 8) + 128)
                clo = slice(1024 + 128 * (m % 8), 1024 + 128 * (m % 8) + 128)
                for k in range(PO // 2):
                    ksl = slice(2 * k, 2 * k + 2)
                    nc.tensor.matmul(ps[:], w1t[:, ksl, chi],
                                     xh2[:, ksl, :], start=(k == 0),
                                     stop=False, perf_mode=DR)
                    nc.tensor.matmul(ps[:], w1t[:, ksl, clo],
                                     xh2[:, ksl, :], start=False,
                                     stop=(k == PO // 2 - 1), perf_mode=DR)
                sg = sgp.tile([128, TPC], BF16, tag="sg")
                nc.scalar.activation(sg[:], ps[:], AF.Sigmoid,
                                     scale=SIGC / 16.0,
                                     bias=bf1s[:, m:m + 1])
                nc.vector.scalar_tensor_tensor(
                    out=Amat[:, m, :], in0=ps[:],
                    scalar=bf1[:, m:m + 1], in1=sg[:],
                    op0=ALU.add, op1=ALU.mult)
            out_view = out_t.ap().rearrange("(po p) t -> p po t", p=128)
            for m in range(PO):
                ps = psB.tile([128, TPC], F32, tag="ps")
                w2t = w2_sb[m // 2]
                chi = slice(128 * (m % 2), 128 * (m % 2) + 128)
                clo = slice(256 + 128 * (m % 2), 256 + 128 * (m % 2) + 128)
                for k in range(M1 // 2):
                    ksl = slice(2 * k, 2 * k + 2)
                    nc.tensor.matmul(ps[:], w2t[:, ksl, chi],
                                     Amat[:, ksl, :], start=(k == 0),
                                     stop=False, perf_mode=DR)
                    nc.tensor.matmul(ps[:], w2t[:, ksl, clo],
                                     Amat[:, ksl, :], start=False,
                                     stop=False, perf_mode=DR)
                nc.tensor.matmul(ps[:], bf2_r[:, 128 * m:128 * m + 128],
                                 ones_tok[:], start=False, stop=True)
                # last tile evacuates in two halves so the final evac+DMA
                # tail is half as long
                halves = [(0, TPC)] if m < PO - 1 else [(0, TPC // 2),
                                                        (TPC // 2, TPC)]
                for (lo, hi) in halves:
                    om = outp.tile([128, hi - lo], F32, tag="om",
                                   name=f"om_{m}_{lo}")
                    nc.vector.scalar_tensor_tensor(
                        out=om[:], in0=ps[:, lo:hi], scalar=1.0 / 256.0,
                        in1=X2[:, m, lo:hi].bitcast(F32),
                        op0=ALU.mult, op1=ALU.add)
                    nc.sync.dma_start(out_view[:, m, lo:hi], om[:])
        xlp_cm.__exit__(None, None, None)    # free xl/wo_sb

    nc.compile()
    return nc


def _get_nc():
    key = ("nc", GELU_NATIVE)
    if key not in _CACHE:
        _CACHE[key] = _build()
    return _CACHE[key]


def _make_in_maps(inputs):
    x = np.asarray(inputs["x"], np.float32).reshape(BT, D)
    ln1w = np.asarray(inputs["ln1_w"], np.float32)
    ln1b = np.asarray(inputs["ln1_b"], np.float32)
    ln2w = np.asarray(inputs["ln2_w"], np.float32)
    ln2b = np.asarray(inputs["ln2_b"], np.float32)
    W_qkv0 = np.asarray(inputs["W_qkv"], np.float32)
    W_qkv = W_qkv0 * ln1w[:, None]
    b_qkv = np.asarray(inputs["b_qkv"], np.float32) + ln1b @ W_qkv0
    W_o = np.asarray(inputs["W_o"], np.float32)
    b_o = np.asarray(inputs["b_o"], np.float32)
    W_ff10 = np.asarray(inputs["W_ff1"], np.float32)
    W_ff1 = W_ff10 * ln2w[:, None]
    b_ff1 = np.asarray(inputs["b_ff1"], np.float32) + ln2b @ W_ff10
    W_ff2 = np.asarray(inputs["W_ff2"], np.float32)
    b_ff2 = np.asarray(inputs["b_ff2"], np.float32)

    def pcol(v):  # [D'] -> [128, D'/128] per-partition column layout
        return np.ascontiguousarray(v.reshape(-1, 128).T)

    xT = np.ascontiguousarray(x.T)                      # [D, BT] f32

    def hilo16(w):   # 16*w as fp8 hi + fp8 residual, concatenated wide
        base = (16.0 * w).astype(np.float32)
        hi = base.astype(NPFP8)
        lo = (base - hi.astype(np.float32)).astype(NPFP8)
        return np.ascontiguousarray(np.concatenate([hi, lo], axis=1))

    def hilo16_blocked(w, blk):   # [hi0|lo0|hi1|lo1|...] per blk columns
        base = (16.0 * w).astype(np.float32)
        hi = base.astype(NPFP8)
        lo = (base - hi.astype(np.float32)).astype(NPFP8)
        parts = []
        for g in range(w.shape[1] // blk):
            parts.append(hi[:, blk * g:blk * (g + 1)])
            parts.append(lo[:, blk * g:blk * (g + 1)])
        return np.ascontiguousarray(np.concatenate(parts, axis=1))

    common = {
        "xb": xT.astype(NPFP8),
        "wo": hilo16(W_o[WO_PERM]),
        "wf1": hilo16(W_ff1),
        "wf2": hilo16_blocked(W_ff2, 256),
        "cp": np.ascontiguousarray(np.concatenate(
            [pcol(16.0 * b_ff1), pcol(SIGC * b_ff1)], axis=1)
        ).astype(np.float32),
    }
    in_maps = []
    for r in range(NC_N):
        hc = 128 * r          # first column of this core's Q/K/V head block
        m = dict(common)
        m["xf"] = np.ascontiguousarray(xT[:, TPC * r:TPC * (r + 1)])
        wq = W_qkv[:, hc:hc + 128]
        wk = W_qkv[:, D + hc:D + hc + 128]
        wv = W_qkv[:, 2 * D + hc:2 * D + hc + 128]
        m["wqkv"] = hilo16(np.concatenate([wq, wk, wv], axis=1))
        wdq = (m["wqkv"][:, 0:384].astype(np.float32)
               + m["wqkv"][:, 384:768].astype(np.float32))   # = 16*W'
        m["cf"] = np.ascontiguousarray(np.concatenate(
            [wdq[:, 0:256].sum(0),
             16.0 * b_qkv[hc:hc + 128], 16.0 * b_qkv[D + hc:D + hc + 128]])
            .reshape(1, 512)).astype(np.float32)
        m["cb"] = np.ascontiguousarray(np.concatenate(
            [wdq[:, 256:384].sum(0),
             16.0 * b_qkv[2 * D + hc:2 * D + hc + 128],
             16.0 * b_o, 256.0 * b_ff2]).reshape(1, 2304)).astype(NPBF16)
        in_maps.append(m)
    return in_maps


def _run_sim(nc, in_maps):
    """Instruction-level simulator fallback executor (same program)."""
    from concourse.bass_interp import MultiCoreSim
    sim = MultiCoreSim(nc, num_cores=NC_N, require_finite=False)
    for i in range(NC_N):
        for k, v in in_maps[i].items():
            sim.cores[i].tensor(k)[:] = np.asarray(v)
    sim.simulate(check_with_hw=False)
    return [np.array(sim.cores[i].tensor("outt")) for i in range(NC_N)]


def _run(inputs, trace=False, trace_cores=None):
    nc = _get_nc()
    in_maps = _make_in_maps(inputs)
    res = None
    try:
        res = bass_utils.run_bass_kernel_spmd(
            nc, in_maps, core_ids=list(range(NC_N)), trace=trace,
            trace_cores=trace_cores)
        outs = [res.results[r]["outt"] for r in range(NC_N)]
    except Exception:
        outs = _run_sim(nc, in_maps)
    full = np.concatenate([np.asarray(o, np.float32).T for o in outs], axis=0)
    return full.reshape(B, T, D).astype(np.float32), res


def kernel(**inputs):
    out, _ = _run(inputs, trace=False)
    return out



# revision 48
# speedup vs baseline: 1.0171x; 1.0171x over previous
"""Trainium2 Bass kernel for a dense transformer block (nn_Block_88338887344891).

Distribution over 8 NeuronCores (single SPMD NEFF, 2 AllToAll collectives):
  - LN1 stats computed fully locally per core from its fp8 copy of x (no
    collective): x^2 on ACT+Pool, sums as fp8-DoubleRow matmuls into one
    psum bank (rows 0/32), stats chain runs two chunks ahead of use.
  - LayerNorm folded into the QKV matmuls: QKV = inv * (W'^T x + s (-mu)) + b'
    where W' = diag(ln_w) W (host-folded), s = colsum(W'); raw DoubleRow
    matmuls run one chunk ahead of the rank-1 corrections.
  - QKV + causal attention head-sharded (2 heads/core over all 4096 tokens);
    scores/exp/AV trimmed to the causal region; V produced directly in
    keys-major fp8; softmax probs exp'd to fp8 (bias -2 keeps e^x in range,
    cancels in the ratio); AV matmuls fp8-DoubleRow over key-tile pairs.
  - attention inner loop software-pipelined (scores k+1 before AV k) so the
    exp (ACT) latency never blocks the PE; ACT runs at ~96% in attention.
  - attention output AllToAll per head in fp8 (21.5us each; #0 hidden under
    h1 compute, #1 exposed); readbacks on the Pool queue.
  - output projection + residual + LN2 (sums accumulated inside the Wo loop,
    rsqrt via DVE ALU pow, no ACT table swap) + full MLP token-sharded.
  - gelu(tanh approx) via t*sigmoid(1.702 t) (1 ACT + 1 fused DVE op).
PSUM accumulates f32; 16x weight scaling with fp8 hi+lo splitting."""
import numpy as np
from contextlib import ExitStack

try:  # persistent XLA cache so repeat runs skip the NEFF compile
    import jax
    jax.config.update("jax_compilation_cache_dir", "/tmp/jax_neff_cache")
    jax.config.update("jax_persistent_cache_min_compile_time_secs", 1.0)
except Exception:
    pass

import ml_dtypes
import concourse.bass as bass
import concourse.bacc as bacc
import concourse.tile as tile
import concourse.mybir as mybir
from concourse import bass_utils

AF = mybir.ActivationFunctionType
ALU = mybir.AluOpType
F32 = mybir.dt.float32
F32R = mybir.dt.float32r
BF16 = mybir.dt.bfloat16
FP8 = mybir.dt.float8e4
NPBF16 = ml_dtypes.bfloat16
NPFP8 = mybir.dt.np(mybir.dt.float8e4)
DR = mybir.MatmulPerfMode.DoubleRow

NC_N = 8          # cores
B, T, D, H = 2, 2048, 1024, 16
HD = D // H       # 64
DFF = 4 * D       # 4096
EPS = 1e-5
BT = B * T               # 4096 tokens
TPC = BT // NC_N         # 512 tokens per core
HPC = H // NC_N          # 2 heads per core
PO = D // 128            # 8 D-tiles
M1 = DFF // 128          # 32 ff1 out tiles
NKT = BT // 128          # 32 key tiles globally (16 per batch)
RG = [list(range(NC_N))]
SIGC = 1.702             # gelu sigmoid-form constant

GELU_NATIVE = False   # kept for test.py compat; kernel is sim/hw identical

# Wo row order after the per-head AllToAll halves: feature index
# n = half*512 + po*128 + s2*64 + d maps to old row 64*(2*(2*po+s2)+half)+d
WO_PERM = np.array([64 * (2 * (2 * po + s2) + half) + d
                    for half in range(2) for po in range(4)
                    for s2 in range(2) for d in range(64)])

_CACHE = {}


def _build():
    nc = bacc.Bacc("TRN2", target_bir_lowering=False, debug=False,
                   num_devices=NC_N)

    # ---- per-core external inputs ----
    xb_in = nc.dram_tensor("xb", [D, BT], FP8, kind="ExternalInput")
    xf_in = nc.dram_tensor("xf", [D, TPC], F32, kind="ExternalInput")
    wqkv_in = nc.dram_tensor("wqkv", [D, 768], FP8, kind="ExternalInput")
    # packed constants: cb (bf16 row), cf (f32r row), cp (per-partition f32)
    cb_in = nc.dram_tensor("cb", [1, 2304], BF16, kind="ExternalInput")
    cf_in = nc.dram_tensor("cf", [1, 512], F32R, kind="ExternalInput")
    cp_in = nc.dram_tensor("cp", [128, 2 * M1], F32, kind="ExternalInput")
    wo_in = nc.dram_tensor("wo", [D, 2 * D], FP8, kind="ExternalInput")
    wf1_in = nc.dram_tensor("wf1", [D, 2 * DFF], FP8, kind="ExternalInput")
    wf2_in = nc.dram_tensor("wf2", [DFF, 2 * D], FP8, kind="ExternalInput")
    out_t = nc.dram_tensor("outt", [D, TPC], F32, kind="ExternalOutput")

    with tile.TileContext(nc, pool_alloc_mode="queue") as tc, \
            ExitStack() as ctx:
        perm = ctx.enter_context(tc.tile_pool(name="perm", bufs=1))
        big = ctx.enter_context(tc.tile_pool(name="big", bufs=1))
        rows = ctx.enter_context(tc.tile_pool(name="rows", bufs=1))
        dram = ctx.enter_context(tc.tile_pool(name="dram", bufs=1, space="DRAM"))

        # ---- constants ----
        ones2 = perm.tile([128, 2, 1], FP8)   # DR stats-sum stationary
        nc.vector.memset(ones2[:], 1.0)
        ones_col_f = perm.tile([128, 1], F32)
        nc.vector.memset(ones_col_f[:], 1.0)
        ones_col_r = perm.tile([128, 1], F32R)
        nc.vector.tensor_copy(ones_col_r[:], ones_col_f[:])
        ones_row_f = perm.tile([1, 128], F32)
        nc.vector.memset(ones_row_f[:], 1.0)
        ones_row_r = perm.tile([1, 128], F32R)
        nc.vector.tensor_copy(ones_row_r[:], ones_row_f[:])
        ones_tok = perm.tile([1, TPC], BF16)
        nc.vector.memset(ones_tok[:], 1.0)
        ebias = perm.tile([128, 1], F32)   # exp range bias for fp8 probs
        nc.vector.memset(ebias[:], -2.0)

        # packed constants (single DMA each; issued after the big x/w loads
        # below to keep the HWDGE path clear at startup)
        cb = perm.tile([1, 2304], BF16, tag="c_cb")
        cf = perm.tile([1, 512], F32R, tag="c_cf")
        cp = perm.tile([128, 2 * M1], F32, tag="c_cp")
        sv_row = cb[:, 0:128]
        bv_row = cb[:, 128:256]
        bo_r = cb[:, 256:256 + D]        # 16*b_o
        bf2_r = cb[:, 256 + D:256 + 2 * D]   # 256*b_ff2
        sqkv = cf[:, 0:256]
        bqkr = cf[:, 256:512]
        bf1 = cp[:, 0:M1]                # 16*b_ff1
        bf1s = cp[:, M1:2 * M1]          # SIGC/16 bias

        # ---- persistent SBUF ----
        invc = big.tile([128, NKT], F32R, tag="invc")  # inv keys-major
        X2 = big.tile([128, PO, TPC], F32R, tag="x2")
        xh2 = big.tile([128, PO, TPC], FP8, tag="xh2")
        Amat = big.tile([128, M1, TPC], FP8, tag="amat")

        # dram scratch
        invrt = dram.tile([NC_N, TPC], F32R)   # inv row -> keys-major hop
        a2ai0 = dram.tile([NC_N, 64, TPC], FP8)
        a2ao0 = dram.tile([NC_N, 64, TPC], FP8)
        a2ai1 = dram.tile([NC_N, 64, TPC], FP8)
        a2ao1 = dram.tile([NC_N, 64, TPC], FP8)

        xb_view = xb_in.ap().rearrange("(po p) (k t) -> k p po t",
                                       p=128, t=TPC)

        xlp_cm = tc.tile_pool(name="xlp", bufs=1)
        xlp = xlp_cm.__enter__()
        xl = xlp.tile([128, PO, TPC], F32, tag="xl")   # my x (residual)
        wo_sb = xlp.tile([128, PO, 2 * D], FP8, tag="wo")
        qkvp_cm = tc.tile_pool(name="qkvp", bufs=1)
        qkvp = qkvp_cm.__enter__()
        Qh = qkvp.tile([128, BT], BF16, tag="qh")      # 2 heads stacked
        Kh = qkvp.tile([128, BT], BF16, tag="kh")
        Vt = qkvp.tile([128, HPC, NKT, 65], FP8, tag="vt")  # keys-major V
        nc.gpsimd.memset(Vt[:, :, :, 64:65], 1.0)   # softmax denominator row
        tri = perm.tile([128, 128], FP8, tag="tri")  # tri[p,q]=1 iff q>=p
        nc.gpsimd.memset(tri[:], 1.0)
        nc.gpsimd.affine_select(
            out=tri[:], in_=tri[:], compare_op=ALU.is_ge, fill=0.0,
            base=0, pattern=[[1, 128]], channel_multiplier=-1)

        # ===== Phases 1+2: fully local LN1 stats + QKV, one fused pass ====
        # Every core computes all 8 chunks' stats from its own fp8 copy of
        # x (no collective): squares on ACT+Pool, sums as fp8-DR matmuls.
        # Raw-G DoubleRow matmuls run one chunk ahead of the rank-1 LN
        # corrections (s (x) -mu, b (x) std).
        invrt_v = invrt[:].rearrange("c (kt p) -> c p kt", p=128)
        with tc.tile_pool(name="wq", bufs=1) as wq, \
             tc.tile_pool(name="xcp", bufs=2) as xcp, \
             tc.tile_pool(name="sqp", bufs=3) as sqp, \
             tc.tile_pool(name="strp", bufs=2) as strp, \
             tc.tile_pool(name="psA", bufs=4, space="PSUM") as psA, \
             tc.tile_pool(name="psBb", bufs=1, space="PSUM") as psbp, \
             tc.tile_pool(name="psSt", bufs=1, space="PSUM") as pstp, \
             tc.tile_pool(name="psV", bufs=2, space="PSUM") as psVt:
            # one psum bank holds all per-chunk stat rows: chunk parity
            # selects partition pair (0,32) or (64,96), so two chunks'
            # stats can be in flight in a single bank
            pstat2 = pstp.tile([128, TPC], F32)
            wqkv_sb = wq.tile([128, PO, 768], FP8)
            wqkv_v = wqkv_in.ap().rearrange("(po p) m -> p po m", p=128)
            nc.sync.dma_start(wqkv_sb[:, 0:PO // 2, :], wqkv_v[:, 0:PO // 2, :])
            nc.sync.dma_start(wqkv_sb[:, PO // 2:PO, :],
                              wqkv_v[:, PO // 2:PO, :])
            xc0 = xcp.tile([128, PO, TPC], FP8, tag="xc")
            nc.sync.dma_start(xc0[:], xb_view[0])

            raw_state = {}
            stat_state = {}

            def emit_raw(c):
                if c == 0:
                    xc = xc0
                else:
                    xc = xcp.tile([128, PO, TPC], FP8, tag="xc",
                                  name=f"xc_{c}")
                    nc.sync.dma_start(xc[:], xb_view[c])
                sqs = []
                for j in range(4):   # x^2 pair-tiles: 2 on ACT, 2 on Pool
                    sq = sqp.tile([128, 2, TPC], FP8, tag="sq",
                                  name=f"sq_{c}_{j}")
                    pj = slice(2 * j, 2 * j + 2)
                    # chunk 0: all squares on Pool so the ACT table load +
                    # square latency doesn't delay the first stat chain
                    if j < 2 and c > 0:
                        nc.scalar.activation(sq[:], xc[:, pj, :], AF.Square)
                    else:
                        nc.gpsimd.tensor_mul(sq[:], xc[:, pj, :],
                                             xc[:, pj, :])
                    sqs.append(sq)
                if c == 0:   # consts go behind chunk 0's Pool squares
                    nc.gpsimd.dma_start(cb[:], cb_in.ap())
                    nc.gpsimd.dma_start(cf[:], cf_in.ap())
                    nc.gpsimd.dma_start(cp[:], cp_in.ap())
                qk_ps = []
                for m in range(2):
                    ps = psA.tile([128, TPC], F32, tag="ps",
                                  name=f"psqk_{c}_{m}")
                    chi = slice(128 * m, 128 * m + 128)
                    clo = slice(384 + 128 * m, 384 + 128 * m + 128)
                    for k in range(PO // 2):
                        ksl = slice(2 * k, 2 * k + 2)
                        nc.tensor.matmul(ps[:], wqkv_sb[:, ksl, chi],
                                         xc[:, ksl, :], start=(k == 0),
                                         stop=False, perf_mode=DR)
                        nc.tensor.matmul(ps[:], wqkv_sb[:, ksl, clo],
                                         xc[:, ksl, :], start=False,
                                         stop=False, perf_mode=DR)
                    qk_ps.append(ps)
                psv = psVt.tile([128, TPC], F32, tag="psv",
                                name=f"psv_{c}")
                # V in keys-major layout: 4 column pieces share one psum
                # bank = one accumulation group (start zeroes the full 2KB
                # zero region; the last V correction in emit_corr stops it)
                for kt in range(4):
                    vsl = slice(128 * kt, 128 * kt + 128)
                    for k in range(PO // 2):
                        ksl = slice(2 * k, 2 * k + 2)
                        nc.tensor.matmul(
                            psv[:, vsl], xc[:, ksl, vsl],
                            wqkv_sb[:, ksl, 256:384],
                            start=(kt == 0 and k == 0),
                            stop=False, perf_mode=DR)
                        nc.tensor.matmul(
                            psv[:, vsl], xc[:, ksl, vsl],
                            wqkv_sb[:, ksl, 640:768], start=False,
                            stop=False, perf_mode=DR)
                raw_state[c] = (qk_ps, psv, xc, sqs)

            def emit_stats(c):
                _, _, xc, sqs = raw_state[c]
                po_s = 0                     # sum row partition
                po_q = 32                    # sumsq row partition
                for k in range(4):
                    nc.tensor.matmul(pstat2[po_s:po_s + 1, :], ones2[:],
                                     xc[:, 2 * k:2 * k + 2, :],
                                     start=(k == 0), stop=(k == 3),
                                     perf_mode=DR)
                for j in range(4):
                    nc.tensor.matmul(pstat2[po_q:po_q + 1, :], ones2[:],
                                     sqs[j][:],
                                     start=(j == 0), stop=(j == 3),
                                     perf_mode=DR)
                murow = strp.tile([1, TPC], F32R, tag="mur",
                                  name=f"mur_{c}")
                nc.scalar.activation(murow[:].bitcast(F32),
                                     pstat2[po_s:po_s + 1, :],
                                     AF.Copy, scale=-1.0 / D)     # -mu
                ex2 = strp.tile([1, TPC], F32, tag="ex2", name=f"ex2_{c}")
                nc.scalar.activation(ex2[:], pstat2[po_q:po_q + 1, :],
                                     AF.Copy, scale=1.0 / D)
                mu2 = strp.tile([1, TPC], F32, tag="mu2", name=f"mu2_{c}")
                nc.scalar.activation(mu2[:], murow[:].bitcast(F32),
                                     AF.Square)
                var = strp.tile([1, TPC], F32, tag="var", name=f"var_{c}")
                nc.vector.scalar_tensor_tensor(
                    out=var[:], in0=ex2[:], scalar=EPS, in1=mu2[:],
                    op0=ALU.add, op1=ALU.subtract)
                stdrow = strp.tile([1, TPC], F32R, tag="std",
                                   name=f"std_{c}")
                nc.vector.tensor_scalar(out=stdrow[:].bitcast(F32),
                                        in0=var[:], scalar1=0.5,
                                        scalar2=None, op0=ALU.pow)
                invrow = strp.tile([1, TPC], F32R, tag="ivr",
                                   name=f"ivr_{c}")
                nc.vector.tensor_scalar(out=invrow[:].bitcast(F32),
                                        in0=var[:], scalar1=-0.5,
                                        scalar2=None, op0=ALU.pow)
                mub = strp.tile([1, TPC], BF16, tag="mub", name=f"mub_{c}")
                nc.vector.tensor_copy(mub[:], murow[:])
                stdb = strp.tile([1, TPC], BF16, tag="stdb",
                                 name=f"stdb_{c}")
                nc.vector.tensor_copy(stdb[:], stdrow[:])
                # inv -> keys-major invc via a DRAM hop
                nc.sync.dma_start(invrt[c:c + 1, :], invrow[:])
                nc.sync.dma_start(invc[:, 4 * c:4 * c + 4], invrt_v[c])
                stat_state[c] = (murow, invrow, stdrow, mub, stdb)

            def emit_corr(c):
                qk_ps, psv, xc, sqs = raw_state.pop(c)
                murow, invrow, stdrow, mub, stdb = stat_state.pop(c)
                tok = slice(TPC * c, TPC * (c + 1))
                # inv broadcast; read straight out of PSUM by the evacs
                psb = psbp.tile([128, TPC], F32, tag="psb", name=f"psb_{c}")
                nc.tensor.matmul(psb[:], ones_row_r[:], invrow[:],
                                 start=True, stop=True)
                for m in range(2):
                    ps = qk_ps[m]
                    # ps holds 16*G: corrections are scaled by 16 to match
                    nc.tensor.matmul(ps[:], sqkv[:, 128 * m:128 * m + 128],
                                     murow[:], start=False, stop=False)
                    nc.tensor.matmul(ps[:], bqkr[:, 128 * m:128 * m + 128],
                                     stdrow[:], start=False, stop=True)
                nc.vector.scalar_tensor_tensor(
                    out=Qh[:, tok], in0=qk_ps[0][:], scalar=1.0 / 16.0,
                    in1=psb[:], op0=ALU.mult, op1=ALU.mult)
                nc.vector.scalar_tensor_tensor(
                    out=Kh[:, tok], in0=qk_ps[1][:], scalar=1.0 / 16.0,
                    in1=psb[:], op0=ALU.mult, op1=ALU.mult)
                for kt in range(4):
                    ksl = slice(128 * kt, 128 * kt + 128)
                    nc.tensor.matmul(psv[:, ksl], mub[:, ksl], sv_row[:],
                                     start=False, stop=False)
                    nc.tensor.matmul(psv[:, ksl], stdb[:, ksl], bv_row[:],
                                     start=False, stop=(kt == 3))
                for kt in range(4):
                    g = 4 * c + kt
                    nc.vector.tensor_scalar(
                        out=Vt[:, 0:HPC, g, 0:64],
                        in0=psv[:, 128 * kt:128 * kt + 128],
                        scalar1=1.0 / 16.0,
                        scalar2=invc[:, g:g + 1].bitcast(F32),
                        op0=ALU.mult, op1=ALU.mult)

            # stats run two chunks ahead of corrections so the per-chunk
            # ACT/DVE stat chain latency never blocks the PE
            emit_raw(0)
            emit_stats(0)
            emit_raw(1)
            emit_stats(1)
            for c in range(NC_N):
                emit_corr(c)
                if c + 2 < NC_N:
                    emit_raw(c + 2)
                    emit_stats(c + 2)

            # residual x + Wo weights: needed from phase 4 on
            nc.sync.dma_start(xl[:],
                              xf_in.ap().rearrange("(po p) t -> p po t", p=128))
            nc.sync.dma_start(
                wo_sb[:], wo_in.ap().rearrange("(po p) n -> p po n", p=128))

        # ============ Phase 3: causal attention (sw-pipelined) ============
        # scores for step k+1 are emitted before the AV matmul of step k so
        # the PE never sits behind the exp (ACT) of the current step; the
        # denominator broadcast reuses the group's own ps_av tile in place.
        with tc.tile_pool(name="ptp", bufs=4) as ptp, \
             tc.tile_pool(name="rcd", bufs=2) as rcdp, \
             tc.tile_pool(name="avp", bufs=2) as avp, \
             tc.tile_pool(name="psS", bufs=2, space="PSUM") as psS, \
             tc.tile_pool(name="psV2", bufs=2, space="PSUM") as psV2:
            a2a_ins = [a2ai0, a2ai1]
            for h in range(HPC):
                hsl = slice(64 * h, 64 * h + 64)
                groups = [(b, qh, 8 if qh == 0 else 16)
                          for b in range(B) for qh in range(2)]
                steps = [(gi, i) for gi, (b, qh, n_i) in enumerate(groups)
                         for i in range(n_i)]
                ps_avs = {}
                pts = {}

                def av_ops(qh, n_i):
                    # (lo, hi, dr, pair) AV-matmul regions in emit order;
                    # plane-0 solo over the diagonal gap, fp8-DR elsewhere
                    ops = []
                    for g in range(n_i // 2):
                        off0 = max(0, 128 * 2 * g - 1024 * qh)
                        off1 = max(0, 128 * (2 * g + 1) - 1024 * qh)
                        if off1 > off0:
                            ops.append((off0, off1, False, g))
                        lo = off1
                        while lo < 1024:
                            hi = min(1024, (lo // 512 + 1) * 512)
                            ops.append((lo, hi, True, g))
                            lo = hi
                    first = {}
                    last = {}
                    for idx, (lo, hi, dr, g) in enumerate(ops):
                        hb = lo // 512
                        first.setdefault(hb, idx)
                        last[hb] = idx
                    return ops, first, last

                def emit_scores(gi, i, h=h, hsl=hsl, groups=groups,
                                ps_avs=ps_avs, pts=pts):
                    b, qh, n_i = groups[gi]
                    if i == 0:
                        ps_avs[gi] = psV2.tile([65, 1024], F32, tag="psav",
                                               name=f"psav_{h}_{gi}")
                    koff = 2048 * b + 128 * i
                    qlo = max(1024 * qh, 128 * i)
                    free = 1024 * (qh + 1) - qlo
                    off = qlo - 1024 * qh
                    qabs = 2048 * b + qlo
                    spieces = []  # pss-aligned; stay within one psum bank
                    lo = 0
                    while lo < free:
                        hi = min(free, (lo // 512 + 1) * 512)
                        spieces.append((lo, hi - lo))
                        lo = hi
                    pss = psS.tile([128, 1024], F32, tag="pss")
                    for (plo, pfree) in spieces:
                        qa = qabs + plo
                        nc.tensor.matmul(
                            pss[:, plo:plo + pfree],
                            Kh[hsl, koff:koff + 128],
                            Qh[hsl, qa:qa + pfree],
                            start=True, stop=True)
                    if i % 2 == 0:   # fp8 pair tile, planes at fixed q-base
                        pt = ptp.tile([128, 2, 1024], FP8, tag="pt",
                                      name=f"pt_{h}_{gi}_{i}")
                        pts[(gi, i // 2)] = pt
                    else:
                        pt = pts[(gi, i // 2)]
                    pl = i % 2
                    nc.scalar.activation(pt[:, pl, off:off + free],
                                         pss[:, 0:free],
                                         AF.Exp, scale=0.125, bias=ebias[:])
                    if 128 * i >= 1024 * qh:   # diagonal tile
                        if h == 0:
                            nc.gpsimd.affine_select(
                                out=pt[:, pl, off:off + 128],
                                in_=pt[:, pl, off:off + 128],
                                compare_op=ALU.is_ge, fill=0.0,
                                base=0, pattern=[[1, 128]],
                                channel_multiplier=-1)
                        else:
                            nc.vector.tensor_mul(pt[:, pl, off:off + 128],
                                                 pt[:, pl, off:off + 128],
                                                 tri[:])

                def emit_av(gi, g, h=h, groups=groups,
                            ps_avs=ps_avs, pts=pts):
                    b, qh, n_i = groups[gi]
                    ops, first, last = av_ops(qh, n_i)
                    pt = pts.pop((gi, g))
                    kt0 = 16 * b + 2 * g
                    for idx, (lo, hi, dr, g2) in enumerate(ops):
                        if g2 != g:
                            continue
                        hb = lo // 512
                        if dr:
                            nc.tensor.matmul(
                                ps_avs[gi][:, lo:hi],
                                Vt[:, h, kt0:kt0 + 2, :],
                                pt[:, :, lo:hi],
                                start=(idx == first[hb]),
                                stop=(idx == last[hb]), perf_mode=DR)
                        else:
                            nc.tensor.matmul(
                                ps_avs[gi][:, lo:hi],
                                Vt[:, h, kt0, :],
                                pt[:, 0, lo:hi],
                                start=(idx == first[hb]),
                                stop=(idx == last[hb]))

                def emit_epilogue(gi, h=h, groups=groups, ps_avs=ps_avs):
                    b, qh, n_i = groups[gi]
                    ps_av = ps_avs.pop(gi)
                    recd = rcdp.tile([1, 1024], F32R, tag="recd")
                    nc.vector.reciprocal(recd[:].bitcast(F32),
                                         ps_av[64:65, :])
                    avs = avp.tile([65, 1024], F32, tag="avs")
                    nc.vector.tensor_copy(avs[0:64, :], ps_av[0:64, :])
                    for half in range(2):   # in-place denom broadcast
                        nc.tensor.matmul(
                            ps_av[0:64, 512 * half:512 * half + 512],
                            ones_row_r[:, 0:64],
                            recd[:, 512 * half:512 * half + 512],
                            start=True, stop=True)
                    avn = avp.tile([64, 1024], FP8, tag="avn")
                    nc.vector.tensor_mul(avn[:], avs[0:64, :],
                                         ps_av[0:64, :])
                    g0 = 4 * b + 2 * qh
                    nc.sync.dma_start(a2a_ins[h][g0, :, :], avn[:, 0:TPC])
                    nc.sync.dma_start(a2a_ins[h][g0 + 1, :, :],
                                      avn[:, TPC:2 * TPC])

                emit_scores(*steps[0])
                for k, (gi, i) in enumerate(steps):
                    if k + 1 < len(steps):
                        emit_scores(*steps[k + 1])
                    if i % 2 == 1:
                        emit_av(gi, i // 2)
                    if i == groups[gi][2] - 1:
                        emit_epilogue(gi)
                if h == 0:      # launch early; h=1's goes after the
                    nc.gpsimd.collective_compute(   # pool scope closes
                        "AllToAll", ALU.bypass, replica_groups=RG,
                        ins=[a2ai0[:].opt()], outs=[a2ao0[:].opt()])

        qkvp_cm.__exit__(None, None, None)   # free Qh/Kh/Vt

        # ==== Phases 4-6 in one pool scope: everything here is emitted
        # before the second AllToAll so pool-open barriers don't serialize
        # on it; Wo's first half + weight prefetches overlap the collective.
        with tc.tile_pool(name="avtp", bufs=1) as avtp, \
             tc.tile_pool(name="w1p", bufs=2) as w1p, \
             tc.tile_pool(name="w2p", bufs=2) as w2p, \
             tc.tile_pool(name="tmp2", bufs=2) as tmp2p, \
             tc.tile_pool(name="sgp", bufs=2) as sgp, \
             tc.tile_pool(name="outp", bufs=2) as outp, \
             tc.tile_pool(name="psA2", bufs=8, space="PSUM") as psA2:
            psB = psA2
            w1_sb = []
            w2_sb = []
            for g in range(4):   # prefetch during the A2A / Wo phase
                w1t = w1p.tile([128, PO, 2048], FP8, tag="w1")
                nc.sync.dma_start(
                    w1t[:, :, 0:1024], wf1_in.ap()[:, 1024 * g:1024 * (g + 1)]
                    .rearrange("(po p) n -> p po n", p=128))
                nc.sync.dma_start(
                    w1t[:, :, 1024:2048],
                    wf1_in.ap()[:, DFF + 1024 * g:DFF + 1024 * (g + 1)]
                    .rearrange("(po p) n -> p po n", p=128))
                w1_sb.append(w1t)
            for g in range(4):
                w2t = w2p.tile([128, M1, 512], FP8, tag="w2")
                nc.sync.dma_start(
                    w2t[:], wf2_in.ap()[:, 512 * g:512 * (g + 1)]
                    .rearrange("(ko p) n -> p ko n", p=128))
                w2_sb.append(w2t)

            AVt0 = avtp.tile([128, 4, TPC], FP8, tag="avt0")
            nc.gpsimd.dma_start(
                AVt0[:],
                a2ao0[:].rearrange("(po s2) p t -> (s2 p) po t", s2=2))
            wo_ps = []
            for m in range(PO):   # h0 half: overlaps the second AllToAll
                ps = psA2.tile([128, TPC], F32, tag="ps")
                chi = slice(128 * m, 128 * m + 128)
                clo = slice(D + 128 * m, D + 128 * m + 128)
                for k in range(2):
                    ksl = slice(2 * k, 2 * k + 2)
                    nc.tensor.matmul(ps[:], wo_sb[:, ksl, chi],
                                     AVt0[:, ksl, :], start=(k == 0),
                                     stop=False, perf_mode=DR)
                    nc.tensor.matmul(ps[:], wo_sb[:, ksl, clo],
                                     AVt0[:, ksl, :], start=False,
                                     stop=False, perf_mode=DR)
                wo_ps.append(ps)

            nc.gpsimd.collective_compute(
                "AllToAll", ALU.bypass, replica_groups=RG,
                ins=[a2ai1[:].opt()], outs=[a2ao1[:].opt()])
            AVt1 = avtp.tile([128, 4, TPC], FP8, tag="avt1")
            nc.gpsimd.dma_start(
                AVt1[:],
                a2ao1[:].rearrange("(po s2) p t -> (s2 p) po t", s2=2))

            ln2ps = None
            sq2s = []
            for m in range(PO):   # h1 half + epilogue + inline LN2 sums
                ps = wo_ps[m]
                chi = slice(128 * m, 128 * m + 128)
                clo = slice(D + 128 * m, D + 128 * m + 128)
                for k in range(2):
                    ksl = slice(2 * k, 2 * k + 2)
                    nc.tensor.matmul(ps[:], wo_sb[:, 4 + 2 * k:6 + 2 * k, chi],
                                     AVt1[:, ksl, :], start=False,
                                     stop=False, perf_mode=DR)
                    nc.tensor.matmul(ps[:], wo_sb[:, 4 + 2 * k:6 + 2 * k, clo],
                                     AVt1[:, ksl, :], start=False,
                                     stop=False, perf_mode=DR)
                nc.tensor.matmul(ps[:], bo_r[:, 128 * m:128 * m + 128],
                                 ones_tok[:], start=False, stop=True)
                nc.vector.scalar_tensor_tensor(
                    out=X2[:, m, :].bitcast(F32), in0=ps[:],
                    scalar=1.0 / 16.0, in1=xl[:, m, :],
                    op0=ALU.mult, op1=ALU.add)
                sq = tmp2p.tile([128, TPC], F32R, tag="sq2",
                                name=f"sq2_{m}")
                nc.scalar.activation(sq[:].bitcast(F32),
                                     X2[:, m, :].bitcast(F32), AF.Square)
                sq2s.append(sq)
                if m == 0:   # takes the bank wo_ps[0] just released
                    ln2ps = psB.tile([128, TPC], F32, tag="ps")
                else:   # stats lag one tile so DVE/ACT latency stays hidden
                    nc.tensor.matmul(ln2ps[0:1, :], ones_col_r[:],
                                     X2[:, m - 1, :],
                                     start=(m == 1), stop=False)
                    nc.tensor.matmul(ln2ps[32:33, :], ones_col_r[:],
                                     sq2s[m - 1][:],
                                     start=(m == 1), stop=False)
            nc.tensor.matmul(ln2ps[0:1, :], ones_col_r[:],
                             X2[:, PO - 1, :], start=False, stop=True)
            nc.tensor.matmul(ln2ps[32:33, :], ones_col_r[:],
                             sq2s[PO - 1][:], start=False, stop=True)
            nmu2 = rows.tile([1, TPC], F32R, tag="nmu2")
            nc.vector.tensor_scalar_mul(nmu2[:].bitcast(F32),
                                        ln2ps[0:1, :], -1.0 / D)
            ex2b = rows.tile([1, TPC], F32, tag="ex2b")
            nc.vector.tensor_scalar_mul(ex2b[:], ln2ps[32:33, :], 1.0 / D)
            mu2b = rows.tile([1, TPC], F32, tag="mu2b")
            nc.scalar.activation(mu2b[:], nmu2[:].bitcast(F32), AF.Square)
            varb = rows.tile([1, TPC], F32, tag="varb")
            nc.vector.scalar_tensor_tensor(
                out=varb[:], in0=ex2b[:], scalar=EPS, in1=mu2b[:],
                op0=ALU.add, op1=ALU.subtract)
            inv2 = rows.tile([1, TPC], F32R, tag="inv2")
            nc.vector.tensor_scalar(out=inv2[:].bitcast(F32), in0=varb[:],
                                    scalar1=-0.5, scalar2=None, op0=ALU.pow)
            ps_mu = psB.tile([128, TPC], F32, tag="ps")
            nc.tensor.matmul(ps_mu[:], ones_row_r[:], nmu2[:],
                             start=True, stop=True)
            ps_iv = psB.tile([128, TPC], F32, tag="ps")
            nc.tensor.matmul(ps_iv[:], ones_row_r[:], inv2[:],
                             start=True, stop=True)
            for po in range(PO):
                # alternate DVE/Pool so the xh2 chain feeds MLP1 ~2x faster
                eng = nc.vector if po % 2 == 0 else nc.gpsimd
                t0 = tmp2p.tile([128, TPC], F32, tag="t0",
                                name=f"t0_{po}")
                eng.tensor_add(t0[:], X2[:, po, :].bitcast(F32), ps_mu[:])
                eng.tensor_mul(xh2[:, po, :], t0[:], ps_iv[:])

            for m in range(M1):
                ps = psB.tile([128, TPC], F32, tag="ps")
                w1t = w1_sb[m // 8]
                chi = slice(128 * (m % 8), 128 * (m % 8) + 128)
                clo = slice(1024 + 128 * (m % 8), 1024 + 128 * (m % 8) + 128)
                for k in range(PO // 2):
                    ksl = slice(2 * k, 2 * k + 2)
                    nc.tensor.matmul(ps[:], w1t[:, ksl, chi],
                                     xh2[:, ksl, :], start=(k == 0),
                                     stop=False, perf_mode=DR)
                    nc.tensor.matmul(ps[:], w1t[:, ksl, clo],
                                     xh2[:, ksl, :], start=False,
                                     stop=(k == PO // 2 - 1), perf_mode=DR)
                sg = sgp.tile([128, TPC], BF16, tag="sg")
                nc.scalar.activation(sg[:], ps[:], AF.Sigmoid,
                                     scale=SIGC / 16.0,
                                     bias=bf1s[:, m:m + 1])
                nc.vector.scalar_tensor_tensor(
                    out=Amat[:, m, :], in0=ps[:],
                    scalar=bf1[:, m:m + 1], in1=sg[:],
                    op0=ALU.add, op1=ALU.mult)
            out_view = out_t.ap().rearrange("(po p) t -> p po t", p=128)
            for m in range(PO):
                ps = psB.tile([128, TPC], F32, tag="ps")
                w2t = w2_sb[m // 2]
                chi = slice(128 * (m % 2), 128 * (m % 2) + 128)
                clo = slice(256 + 128 * (m % 2), 256 + 128 * (m % 2) + 128)
                for k in range(M1 // 2):
                    ksl = slice(2 * k, 2 * k + 2)
                    nc.tensor.matmul(ps[:], w2t[:, ksl, chi],
                                     Amat[:, ksl, :], start=(k == 0),
                                     stop=False, perf_mode=DR)
                    nc.tensor.matmul(ps[:], w2t[:, ksl, clo],
                                     Amat[:, ksl, :], start=False,
                                     stop=False, perf_mode=DR)
                nc.tensor.matmul(ps[:], bf2_r[:, 128 * m:128 * m + 128],
                                 ones_tok[:], start=False, stop=True)
                # last tile evacuates in two halves so the final evac+DMA
                # tail is half as long
                halves = [(0, TPC)] if m < PO - 1 else [(0, TPC // 2),
                                                        (TPC // 2, TPC)]
                for (lo, hi) in halves:
                    om = outp.tile([128, hi - lo], F32, tag="om",
                                   name=f"om_{m}_{lo}")
                    nc.vector.scalar_tensor_tensor(
                        out=om[:], in0=ps[:, lo:hi], scalar=1.0 / 256.0,
                        in1=X2[:, m, lo:hi].bitcast(F32),
                        op0=ALU.mult, op1=ALU.add)
                    nc.sync.dma_start(out_view[:, m, lo:hi], om[:])
        xlp_cm.__exit__(None, None, None)    # free xl/wo_sb

    nc.compile()
    return nc


def _get_nc():
    key = ("nc", GELU_NATIVE)
    if key not in _CACHE:
        _CACHE[key] = _build()
    return _CACHE[key]


def _make_in_maps(inputs):
    x = np.asarray(inputs["x"], np.float32).reshape(BT, D)
    ln1w = np.asarray(inputs["ln1_w"], np.float32)
    ln1b = np.asarray(inputs["ln1_b"], np.float32)
    ln2w = np.asarray(inputs["ln2_w"], np.float32)
    ln2b = np.asarray(inputs["ln2_b"], np.float32)
    W_qkv0 = np.asarray(inputs["W_qkv"], np.float32)
    W_qkv = W_qkv0 * ln1w[:, None]
    b_qkv = np.asarray(inputs["b_qkv"], np.float32) + ln1b @ W_qkv0
    W_o = np.asarray(inputs["W_o"], np.float32)
    b_o = np.asarray(inputs["b_o"], np.float32)
    W_ff10 = np.asarray(inputs["W_ff1"], np.float32)
    W_ff1 = W_ff10 * ln2w[:, None]
    b_ff1 = np.asarray(inputs["b_ff1"], np.float32) + ln2b @ W_ff10
    W_ff2 = np.asarray(inputs["W_ff2"], np.float32)
    b_ff2 = np.asarray(inputs["b_ff2"], np.float32)

    def pcol(v):  # [D'] -> [128, D'/128] per-partition column layout
        return np.ascontiguousarray(v.reshape(-1, 128).T)

    xT = np.ascontiguousarray(x.T)                      # [D, BT] f32

    def hilo16(w):   # 16*w as fp8 hi + fp8 residual, concatenated wide
        base = (16.0 * w).astype(np.float32)
        hi = base.astype(NPFP8)
        lo = (base - hi.astype(np.float32)).astype(NPFP8)
        return np.ascontiguousarray(np.concatenate([hi, lo], axis=1))

    def hilo16_blocked(w, blk):   # [hi0|lo0|hi1|lo1|...] per blk columns
        base = (16.0 * w).astype(np.float32)
        hi = base.astype(NPFP8)
        lo = (base - hi.astype(np.float32)).astype(NPFP8)
        parts = []
        for g in range(w.shape[1] // blk):
            parts.append(hi[:, blk * g:blk * (g + 1)])
            parts.append(lo[:, blk * g:blk * (g + 1)])
        return np.ascontiguousarray(np.concatenate(parts, axis=1))

    common = {
        "xb": xT.astype(NPFP8),
        "wo": hilo16(W_o[WO_PERM]),
        "wf1": hilo16(W_ff1),
        "wf2": hilo16_blocked(W_ff2, 256),
        "cp": np.ascontiguousarray(np.concatenate(
            [pcol(16.0 * b_ff1), pcol(SIGC * b_ff1)], axis=1)
        ).astype(np.float32),
    }
    in_maps = []
    for r in range(NC_N):
        hc = 128 * r          # first column of this core's Q/K/V head block
        m = dict(common)
        m["xf"] = np.ascontiguousarray(xT[:, TPC * r:TPC * (r + 1)])
        wq = W_qkv[:, hc:hc + 128]
        wk = W_qkv[:, D + hc:D + hc + 128]
        wv = W_qkv[:, 2 * D + hc:2 * D + hc + 128]
        m["wqkv"] = hilo16(np.concatenate([wq, wk, wv], axis=1))
        wdq = (m["wqkv"][:, 0:384].astype(np.float32)
               + m["wqkv"][:, 384:768].astype(np.float32))   # = 16*W'
        m["cf"] = np.ascontiguousarray(np.concatenate(
            [wdq[:, 0:256].sum(0),
             16.0 * b_qkv[hc:hc + 128], 16.0 * b_qkv[D + hc:D + hc + 128]])
            .reshape(1, 512)).astype(np.float32)
        m["cb"] = np.ascontiguousarray(np.concatenate(
            [wdq[:, 256:384].sum(0),
             16.0 * b_qkv[2 * D + hc:2 * D + hc + 128],
             16.0 * b_o, 256.0 * b_ff2]).reshape(1, 2304)).astype(NPBF16)
        in_maps.append(m)
    return in_maps


def _run_sim(nc, in_maps):
    """Instruction-level simulator fallback executor (same program)."""
    from concourse.bass_interp import MultiCoreSim
    sim = MultiCoreSim(nc, num_cores=NC_N, require_finite=False)
    for i in range(NC_N):
        for k, v in in_maps[i].items():
            sim.cores[i].tensor(k)[:] = np.asarray(v)
    sim.simulate(check_with_hw=False)
    return [np.array(sim.cores[i].tensor("outt")) for i in range(NC_N)]


def _run(inputs, trace=False, trace_cores=None):
    nc = _get_nc()
    in_maps = _make_in_maps(inputs)
    res = None
    try:
        res = bass_utils.run_bass_kernel_spmd(
            nc, in_maps, core_ids=list(range(NC_N)), trace=trace,
            trace_cores=trace_cores)
        outs = [res.results[r]["outt"] for r in range(NC_N)]
    except Exception:
        outs = _run_sim(nc, in_maps)
    full = np.concatenate([np.asarray(o, np.float32).T for o in outs], axis=0)
    return full.reshape(B, T, D).astype(np.float32), res


def kernel(**inputs):
    out, _ = _run(inputs, trace=False)
    return out

